# revision 32
# baseline (speedup 1.0000x reference)
"""Distributed Trainium2 kernel: relative-position multi-head attention.

B=2, N=2048, DIM=1536, H=8, DK=64, DV=192.

Sharding: one head per core, both batches (8 heads / 8 cores).  Each core
projects q/k/v for its head over all 4096 tokens, runs attention, transposes
its attention output to feature-major, then one 8-core AllToAll exchanges
token-slices: core c ends up with all 8 heads' outputs for flat token rows
[c*512,(c+1)*512) of the flattened [4096, DIM] output, and computes that
slice of the output projection.

Math: rel_k = distances @ W_rel is rank-1, so after relative_shift the
positional logits are s_i*(j-i) with s_i = (q_i*scale+rpb)@w_h.  The -s_i*i
term is constant per softmax row and drops under softmax.  So
logits = (q*scale+rcb)@k^T + s_i*j, realized as 2 extra contraction rows of
the QK^T matmul: [jvec, ones] on the K side and [s_i, -B_i] on the Q side,
where B_i upper-bounds the row max so exp cannot overflow; it cancels
exactly in softmax.  s_i comes free out of the projection via an extra
weight column u = Wq_scaled @ w_h.
"""

import contextlib

import ml_dtypes
import numpy as np

import concourse.bass as bass
import concourse.bacc as bacc_mod
import concourse.mybir as mybir
import concourse.tile as tile
from concourse.bass_utils import run_bass_kernel_spmd
from concourse.masks import make_identity

B, N, DIM, H, DK, DV = 2, 2048, 1536, 8, 64, 192
NCORES = 8
NT = B * N               # 4096 flat tokens
NQ = NT // NCORES        # 512 output rows per core
P = 128
DCH = DIM // P           # 12 projection contraction chunks
NTILE = N // P           # 16 token tiles per batch
IBLK = 512
NIB = N // IBLK          # 4 i-blocks per batch
F32 = mybir.dt.float32
F32R = mybir.dt.float32r
BF16 = mybir.dt.bfloat16
AT = mybir.AluOpType
AF = mybir.ActivationFunctionType
CONTENT_BOUND = 48.0

_CACHE = {}


def _build_body(nc, tc, xT, wqk, wv1, wv2, wo, krows, ccol, bor, out):
    ctx = contextlib.ExitStack()
    with ctx:
        persist = ctx.enter_context(tc.tile_pool(name="persist", bufs=1))

        wqk_sb = persist.tile([P, DCH * P], BF16, tag="wqk")
        wv1_sb = persist.tile([P, DCH * P], BF16, tag="wv1")
        wv2_sb = persist.tile([P, DCH * 65], BF16, tag="wv2")
        wo_sb = persist.tile([P, DCH * DIM], BF16, tag="wo")
        ccol_sb = persist.tile([P, 2], F32, tag="ccol")
        bor_sb = persist.tile([P, DIM], F32, tag="bor")
        ident = persist.tile([P, P], BF16, tag="ident")

        for w_sb, w_dram in ((wqk_sb, wqk), (wv1_sb, wv1), (wv2_sb, wv2)):
            nc.sync.dma_start(out=w_sb[:], in_=w_dram[:])
        nc.sync.dma_start(out=ccol_sb[:], in_=ccol[:])
        make_identity(nc, ident[:])

        # Q'/K' per flat token: rows 0-63 content, 64 ramp (s / jvec), 65 bias (-B / ones)
        QT = persist.tile([67, NT], BF16, tag="QT")
        KT = persist.tile([67, NT], BF16, tag="KT")
        # v token-major per (b, j-tile): [dv(192) | ones]
        vtok = persist.tile([P, B * NTILE * (DV + 1)], BF16, tag="vtok")
        # attention output feature-major (transposed), split 128/64 partitions
        attTa = persist.tile([P, NT], BF16, tag="attTa")
        attTb = persist.tile([64, NT], BF16, tag="attTb")

        for b in range(B):
            bs = b * N
            nc.sync.dma_start(out=KT[64:67, bs:bs + N], in_=krows[:])  # jhi; jlo; ones

        # ---------------- phase 1: projections ----------------
        with tc.tile_pool(name="xch", bufs=8) as xpool, \
             tc.tile_pool(name="prjp", bufs=2, space="PSUM") as ppsum, \
             tc.tile_pool(name="vT", bufs=1) as vtpool, \
             tc.tile_pool(name="vtp", bufs=2, space="PSUM") as vtpsum, \
             tc.tile_pool(name="srow", bufs=1) as spool, \
             tc.tile_pool(name="sdram", bufs=2, space="DRAM") as sdram:

            srow = [spool.tile([1, N], F32, tag=f"srow{b}", name=f"srow{b}") for b in range(B)]

            vTa = vtpool.tile([P, NT], BF16, tag="vTa")
            vTb = vtpool.tile([64, NT], BF16, tag="vTb")

            xc_cache = {}
            for b in range(B):
                for it in range(NIB):
                    bt = b * NIB + it
                    sl = slice(bt * IBLK, (bt + 1) * IBLK)
                    pqk = ppsum.tile([P, IBLK], F32, tag="pqk", name=f"pqk{bt}")
                    pv1 = ppsum.tile([P, IBLK], F32, tag="pv1", name=f"pv1{bt}")
                    pv2 = ppsum.tile([P, IBLK], F32, tag="pv2", name=f"pv2{bt}")
                    for c in range(DCH):
                        if it % 2 == 0:
                            xc2 = xpool.tile([P, 2 * IBLK], BF16, tag="xc", name=f"xc{bt}_{c}")
                            nc.sync.dma_start(
                                out=xc2[:], in_=xT[c * P:(c + 1) * P, bt * IBLK:(bt + 2) * IBLK])
                            xc_cache[c] = xc2
                        mk = dict(start=(c == 0), stop=(c == DCH - 1))
                        xr = xc_cache[c][:, (it % 2) * IBLK:(it % 2 + 1) * IBLK]
                        nc.tensor.matmul(pqk[:], wqk_sb[:, c * P:(c + 1) * P], xr, **mk)
                        nc.tensor.matmul(pv1[:], wv1_sb[:, c * P:(c + 1) * P], xr, **mk)
                        nc.tensor.matmul(pv2[0:65, :], wv2_sb[:, c * 65:(c + 1) * 65], xr, **mk)
                    nc.vector.tensor_scalar_add(QT[0:DK, sl], pqk[0:DK, :], ccol_sb[0:DK, 0:1])
                    nc.vector.tensor_copy(KT[0:DK, sl], pqk[DK:2 * DK, :])
                    nc.vector.tensor_copy(vTa[:, sl], pv1[:])
                    nc.vector.tensor_copy(vTb[:, sl], pv2[0:DK, :])
                    nc.vector.tensor_scalar_add(
                        srow[b][0:1, it * IBLK:(it + 1) * IBLK],
                        pv2[64:65, :], ccol_sb[0:1, 1:2])

                # assemble batch b immediately: v token-major + QT ramp rows
                bs = b * N
                for jt in range(NTILE):
                    bj = b * NTILE + jt
                    tp = vtpsum.tile([P, 2 * P], BF16, tag="vtp", name=f"vtp{bj}")
                    nc.tensor.transpose(tp[:, 0:P], vTa[:, bj * P:(bj + 1) * P], ident[:])
                    nc.tensor.transpose(tp[:, P:P + 64], vTb[:, bj * P:(bj + 1) * P], ident[0:64, 0:64])
                    base = bj * (DV + 1)
                    nc.vector.tensor_copy(vtok[:, base:base + P], tp[:, 0:P])
                    nc.vector.tensor_copy(vtok[:, base + P:base + DV], tp[:, P:P + 64])
                    nc.gpsimd.memset(vtok[:, base + DV:base + DV + 1], 1.0)
                # B = relu(s)*(N-1) + CONTENT_BOUND  (ramp max over j; -s*i dropped)
                t8 = spool.tile([1, N], BF16, tag="bt8", name=f"bt8{b}")
                t1f = spool.tile([1, N], BF16, tag="bt1f", name=f"bt1f{b}")
                tb = spool.tile([1, N], BF16, tag="btb", name=f"btb{b}")
                tr = spool.tile([1, N], F32, tag="btr", name=f"btr{b}")
                nc.vector.tensor_scalar_mul(t8[:], srow[b][:], 8.0)
                nc.vector.tensor_copy(t1f[:], srow[b][:])
                nc.vector.tensor_scalar_max(tr[:], srow[b][:], 0.0)
                nc.vector.tensor_scalar(tb[:], tr[:], -float(N - 1), -CONTENT_BOUND, AT.mult, AT.add)
                qs3 = sdram.tile([3, N], BF16, tag="qs3", name=f"qs3{b}")
                nc.sync.dma_start(out=qs3[0:1, :], in_=t8[:])
                nc.sync.dma_start(out=qs3[1:2, :], in_=t1f[:])
                nc.sync.dma_start(out=qs3[2:3, :], in_=tb[:])
                nc.sync.dma_start(out=QT[64:67, bs:bs + N], in_=qs3[:])

        # ---------------- phase 2: attention ----------------
        with tc.tile_pool(name="et", bufs=2) as epool, \
             tc.tile_pool(name="lg", bufs=2, space="PSUM") as lgp, \
             tc.tile_pool(name="av", bufs=2, space="PSUM") as avp, \
             tc.tile_pool(name="atp", bufs=2, space="PSUM") as atpp, \
             tc.tile_pool(name="rz", bufs=4) as rzpool, \
             tc.tile_pool(name="an", bufs=4) as anpool:
            for b in range(B):
                bs = b * N
                for ib in range(NIB):
                    isl = slice(bs + ib * IBLK, bs + (ib + 1) * IBLK)
                    eT = epool.tile([P, NTILE * IBLK], BF16, tag="eT")
                    for jt in range(NTILE):
                        lg = lgp.tile([P, IBLK], F32, tag="lg")
                        nc.tensor.matmul(
                            lg[:], KT[:, bs + jt * P:bs + (jt + 1) * P],
                            QT[:, isl], start=True, stop=True)
                        nc.scalar.activation(eT[:, jt * IBLK:(jt + 1) * IBLK], lg[:], AF.Exp)
                    for ic in range(IBLK // P):
                        av = avp.tile([P, DV + 1], F32, tag="av")
                        for jt in range(NTILE):
                            nc.tensor.matmul(
                                av[:],
                                eT[:, jt * IBLK + ic * P:jt * IBLK + (ic + 1) * P],
                                vtok[:, (b * NTILE + jt) * (DV + 1):(b * NTILE + jt + 1) * (DV + 1)],
                                start=(jt == 0), stop=(jt == NTILE - 1))
                        rz = rzpool.tile([P, 1], F32, tag="rz")
                        nc.vector.reciprocal(rz[:], av[:, DV:DV + 1])
                        an = anpool.tile([P, DV], BF16, tag="an")
                        nc.vector.tensor_scalar_mul(an[:], av[:, 0:DV], rz[:])
                        # transpose [i, dv] -> feature-major attTa/attTb
                        iabs = bs + ib * IBLK + ic * P
                        ta = atpp.tile([P, P], BF16, tag="ta")
                        nc.tensor.transpose(ta[:], an[:, 0:P], ident[:])
                        nc.vector.tensor_copy(attTa[:, iabs:iabs + P], ta[:])
                        tb = atpp.tile([P, P], BF16, tag="tb")
                        nc.tensor.transpose(tb[0:64, 0:P], an[:, P:DV], ident[:])
                        nc.vector.tensor_copy(attTb[:, iabs:iabs + P], tb[0:64, 0:P])

        # ---------------- phase 3: per-batch AllToAll + output projection ----------------
        # a2a[b]: input [8 dest, 192 dv, 256 rows of batch b] -> core c owns
        # batch-b rows [c*256,(c+1)*256).  out rows: [b0 256 | b1 256].
        HQ = NQ // B  # 256 rows per batch per core
        with tc.tile_pool(name="dram", bufs=1, space="DRAM") as dram, \
             tc.tile_pool(name="gat", bufs=1) as gpool, \
             tc.tile_pool(name="yp", bufs=4, space="PSUM") as ypsum, \
             tc.tile_pool(name="yo", bufs=4) as ypool:
            gatT = gpool.tile([P, DCH * NQ], BF16, tag="gatT")
            nc.sync.dma_start(out=wo_sb[:], in_=wo[:])
            nc.sync.dma_start(out=bor_sb[:], in_=bor[:])

            def do_exchange(b):
                bs = b * N
                a2a_in = dram.tile([NCORES * DV, HQ], BF16, name=f"a2a_in{b}", tag=f"a2a_in{b}")
                a2a_out = dram.tile([NCORES * DV, HQ], BF16, name=f"a2a_out{b}", tag=f"a2a_out{b}")
                nc.sync.dma_start(
                    out=a2a_in[:].rearrange("(d r) n -> r d n", r=DV)[0:P],
                    in_=attTa[:, bs:bs + N].rearrange("p (d n) -> p d n", d=NCORES))
                nc.sync.dma_start(
                    out=a2a_in[:].rearrange("(d r) n -> r d n", r=DV)[P:DV],
                    in_=attTb[:, bs:bs + N].rearrange("p (d n) -> p d n", d=NCORES))
                nc.gpsimd.collective_compute(
                    "AllToAll", AT.bypass,
                    replica_groups=[list(range(NCORES))],
                    ins=[a2a_in[:].opt()], outs=[a2a_out[:].opt()])
                # received [src, dv 192, my 256 rows] = [1536 dv, 256]
                nc.sync.dma_start(
                    out=gatT[:].rearrange("p (c n) -> p c n", c=DCH)[:, :, b * HQ:(b + 1) * HQ],
                    in_=a2a_out[:].rearrange("(c p) n -> p c n", p=P))

            def do_outproj(b, ypsum, ypool):
                for it in range(HQ // P):
                    row = b * HQ + it * P
                    for ot in range(DIM // IBLK):
                        yp = ypsum.tile([P, IBLK], F32, tag="yp")
                        for kc in range(DCH):
                            nc.tensor.matmul(
                                yp[:], gatT[:, kc * NQ + row:kc * NQ + row + P],
                                wo_sb[:, kc * DIM + ot * IBLK:kc * DIM + (ot + 1) * IBLK],
                                start=(kc == 0), stop=(kc == DCH - 1))
                        yo = ypool.tile([P, IBLK], F32, tag="yo")
                        nc.vector.tensor_add(yo[:], yp[:], bor_sb[:, ot * IBLK:(ot + 1) * IBLK])
                        nc.sync.dma_start(
                            out=out[row:row + P, ot * IBLK:(ot + 1) * IBLK], in_=yo[:])

            for b in range(B):
                do_exchange(b)
                do_outproj(b)


# revision 34
# speedup vs baseline: 1.0809x; 1.0809x over previous
"""Distributed Trainium2 kernel: relative-position multi-head attention.

B=2, N=2048, DIM=1536, H=8, DK=64, DV=192.

Sharding: one head per core, both batches (8 heads / 8 cores).  Each core
projects q/k/v for its head over all 4096 tokens, runs attention, transposes
its attention output to feature-major, then one 8-core AllToAll exchanges
token-slices: core c ends up with all 8 heads' outputs for flat token rows
[c*512,(c+1)*512) of the flattened [4096, DIM] output, and computes that
slice of the output projection.

Math: rel_k = distances @ W_rel is rank-1, so after relative_shift the
positional logits are s_i*(j-i) with s_i = (q_i*scale+rpb)@w_h.  The -s_i*i
term is constant per softmax row and drops under softmax.  So
logits = (q*scale+rcb)@k^T + s_i*j, realized as 2 extra contraction rows of
the QK^T matmul: [jvec, ones] on the K side and [s_i, -B_i] on the Q side,
where B_i upper-bounds the row max so exp cannot overflow; it cancels
exactly in softmax.  s_i comes free out of the projection via an extra
weight column u = Wq_scaled @ w_h.
"""

import contextlib

import ml_dtypes
import numpy as np

import concourse.bass as bass
import concourse.bacc as bacc_mod
import concourse.mybir as mybir
import concourse.tile as tile
from concourse.bass_utils import run_bass_kernel_spmd
from concourse.masks import make_identity

B, N, DIM, H, DK, DV = 2, 2048, 1536, 8, 64, 192
NCORES = 8
NT = B * N               # 4096 flat tokens
NQ = NT // NCORES        # 512 output rows per core
P = 128
DCH = DIM // P           # 12 projection contraction chunks
NTILE = N // P           # 16 token tiles per batch
IBLK = 512
NIB = N // IBLK          # 4 i-blocks per batch
F32 = mybir.dt.float32
F32R = mybir.dt.float32r
BF16 = mybir.dt.bfloat16
AT = mybir.AluOpType
AF = mybir.ActivationFunctionType
CONTENT_BOUND = 48.0

_CACHE = {}


def _build_body(nc, tc, xT, wqk, wv1, wv2, wo, krows, ccol, bor, out):
    ctx = contextlib.ExitStack()
    with ctx:
        persist = ctx.enter_context(tc.tile_pool(name="persist", bufs=1))

        wqk_sb = persist.tile([P, DCH * P], BF16, tag="wqk")
        wv1_sb = persist.tile([P, DCH * P], BF16, tag="wv1")
        wv2_sb = persist.tile([P, DCH * 65], BF16, tag="wv2")
        wo_sb = persist.tile([P, DCH * DIM], BF16, tag="wo")
        ccol_sb = persist.tile([P, 2], F32, tag="ccol")
        bor_sb = persist.tile([P, DIM], F32, tag="bor")
        ident = persist.tile([P, P], BF16, tag="ident")

        for w_sb, w_dram in ((wqk_sb, wqk), (wv1_sb, wv1), (wv2_sb, wv2)):
            nc.sync.dma_start(out=w_sb[:], in_=w_dram[:])
        nc.sync.dma_start(out=ccol_sb[:], in_=ccol[:])
        make_identity(nc, ident[:])

        # Q'/K' per flat token: rows 0-63 content, 64 ramp (s / jvec), 65 bias (-B / ones)
        QT = persist.tile([67, NT], BF16, tag="QT")
        KT = persist.tile([67, NT], BF16, tag="KT")
        # v token-major per (b, j-tile): [dv(192) | ones]
        vtok = persist.tile([P, B * NTILE * (DV + 1)], BF16, tag="vtok")
        # attention output feature-major (transposed), split 128/64 partitions
        attTa = persist.tile([P, NT], BF16, tag="attTa")
        attTb = persist.tile([64, NT], BF16, tag="attTb")

        for b in range(B):
            bs = b * N
            nc.sync.dma_start(out=KT[64:67, bs:bs + N], in_=krows[:])  # jhi; jlo; ones

        # ---------------- phase 1: projections ----------------
        with tc.tile_pool(name="xch", bufs=16) as xpool, \
             tc.tile_pool(name="prjp", bufs=2, space="PSUM") as ppsum, \
             tc.tile_pool(name="vT", bufs=1) as vtpool, \
             tc.tile_pool(name="vtp", bufs=2, space="PSUM") as vtpsum, \
             tc.tile_pool(name="srow", bufs=1) as spool, \
             tc.tile_pool(name="sdram", bufs=2, space="DRAM") as sdram:

            srow = [spool.tile([1, N], F32, tag=f"srow{b}", name=f"srow{b}") for b in range(B)]

            vTa = vtpool.tile([P, NT], BF16, tag="vTa")
            vTb = vtpool.tile([64, NT], BF16, tag="vTb")

            xc_cache = {}
            for b in range(B):
                for it in range(NIB):
                    bt = b * NIB + it
                    sl = slice(bt * IBLK, (bt + 1) * IBLK)
                    pqk = ppsum.tile([P, IBLK], F32, tag="pqk", name=f"pqk{bt}")
                    pv1 = ppsum.tile([P, IBLK], F32, tag="pv1", name=f"pv1{bt}")
                    pv2 = ppsum.tile([P, IBLK], F32, tag="pv2", name=f"pv2{bt}")
                    for c in range(DCH):
                        if it % 2 == 0:
                            xc2 = xpool.tile([P, 2 * IBLK], BF16, tag="xc", name=f"xc{bt}_{c}")
                            nc.sync.dma_start(
                                out=xc2[:], in_=xT[c * P:(c + 1) * P, bt * IBLK:(bt + 2) * IBLK])
                            xc_cache[c] = xc2
                        mk = dict(start=(c == 0), stop=(c == DCH - 1))
                        xr = xc_cache[c][:, (it % 2) * IBLK:(it % 2 + 1) * IBLK]
                        nc.tensor.matmul(pqk[:], wqk_sb[:, c * P:(c + 1) * P], xr, **mk)
                        nc.tensor.matmul(pv1[:], wv1_sb[:, c * P:(c + 1) * P], xr, **mk)
                        nc.tensor.matmul(pv2[0:65, :], wv2_sb[:, c * 65:(c + 1) * 65], xr, **mk)
                    nc.vector.tensor_scalar_add(QT[0:DK, sl], pqk[0:DK, :], ccol_sb[0:DK, 0:1])
                    nc.vector.tensor_copy(KT[0:DK, sl], pqk[DK:2 * DK, :])
                    nc.vector.tensor_copy(vTa[:, sl], pv1[:])
                    nc.vector.tensor_copy(vTb[:, sl], pv2[0:DK, :])
                    nc.vector.tensor_scalar_add(
                        srow[b][0:1, it * IBLK:(it + 1) * IBLK],
                        pv2[64:65, :], ccol_sb[0:1, 1:2])

                # assemble batch b immediately: v token-major + QT ramp rows
                bs = b * N
                for jt in range(NTILE):
                    bj = b * NTILE + jt
                    tp = vtpsum.tile([P, 2 * P], BF16, tag="vtp", name=f"vtp{bj}")
                    nc.tensor.transpose(tp[:, 0:P], vTa[:, bj * P:(bj + 1) * P], ident[:])
                    nc.tensor.transpose(tp[:, P:P + 64], vTb[:, bj * P:(bj + 1) * P], ident[0:64, 0:64])
                    base = bj * (DV + 1)
                    nc.vector.tensor_copy(vtok[:, base:base + P], tp[:, 0:P])
                    nc.vector.tensor_copy(vtok[:, base + P:base + DV], tp[:, P:P + 64])
                    nc.gpsimd.memset(vtok[:, base + DV:base + DV + 1], 1.0)
                # B = relu(s)*(N-1) + CONTENT_BOUND  (ramp max over j; -s*i dropped)
                t8 = spool.tile([1, N], BF16, tag="bt8", name=f"bt8{b}")
                t1f = spool.tile([1, N], BF16, tag="bt1f", name=f"bt1f{b}")
                tb = spool.tile([1, N], BF16, tag="btb", name=f"btb{b}")
                tr = spool.tile([1, N], F32, tag="btr", name=f"btr{b}")
                nc.vector.tensor_scalar_mul(t8[:], srow[b][:], 8.0)
                nc.vector.tensor_copy(t1f[:], srow[b][:])
                nc.vector.tensor_scalar_max(tr[:], srow[b][:], 0.0)
                nc.vector.tensor_scalar(tb[:], tr[:], -float(N - 1), -CONTENT_BOUND, AT.mult, AT.add)
                qs3 = sdram.tile([3, N], BF16, tag="qs3", name=f"qs3{b}")
                nc.sync.dma_start(out=qs3[0:1, :], in_=t8[:])
                nc.sync.dma_start(out=qs3[1:2, :], in_=t1f[:])
                nc.sync.dma_start(out=qs3[2:3, :], in_=tb[:])
                nc.sync.dma_start(out=QT[64:67, bs:bs + N], in_=qs3[:])

        # ---------------- phase 2: attention ----------------
        with tc.tile_pool(name="et", bufs=2) as epool, \
             tc.tile_pool(name="lg", bufs=3, space="PSUM") as lgp, \
             tc.tile_pool(name="av", bufs=2, space="PSUM") as avp, \
             tc.tile_pool(name="atp", bufs=1, space="PSUM") as atpp, \
             tc.tile_pool(name="rz", bufs=4) as rzpool, \
             tc.tile_pool(name="an", bufs=4) as anpool:
            for b in range(B):
                bs = b * N
                for ib in range(NIB):
                    isl = slice(bs + ib * IBLK, bs + (ib + 1) * IBLK)
                    eT = epool.tile([P, NTILE * IBLK], BF16, tag="eT")
                    for jt in range(NTILE):
                        lg = lgp.tile([P, IBLK], F32, tag="lg")
                        nc.tensor.matmul(
                            lg[:], KT[:, bs + jt * P:bs + (jt + 1) * P],
                            QT[:, isl], start=True, stop=True)
                        nc.scalar.activation(eT[:, jt * IBLK:(jt + 1) * IBLK], lg[:], AF.Exp)
                    for ic in range(IBLK // P):
                        av = avp.tile([P, DV + 1], F32, tag="av")
                        for jt in range(NTILE):
                            nc.tensor.matmul(
                                av[:],
                                eT[:, jt * IBLK + ic * P:jt * IBLK + (ic + 1) * P],
                                vtok[:, (b * NTILE + jt) * (DV + 1):(b * NTILE + jt + 1) * (DV + 1)],
                                start=(jt == 0), stop=(jt == NTILE - 1))
                        rz = rzpool.tile([P, 1], F32, tag="rz")
                        nc.vector.reciprocal(rz[:], av[:, DV:DV + 1])
                        an = anpool.tile([P, DV], BF16, tag="an")
                        nc.vector.tensor_scalar_mul(an[:], av[:, 0:DV], rz[:])
                        # transpose [i, dv] -> feature-major attTa/attTb
                        iabs = bs + ib * IBLK + ic * P
                        ta = atpp.tile([P, P], BF16, tag="ta")
                        nc.tensor.transpose(ta[:], an[:, 0:P], ident[:])
                        nc.vector.tensor_copy(attTa[:, iabs:iabs + P], ta[:])
                        tb = atpp.tile([P, P], BF16, tag="tb")
                        nc.tensor.transpose(tb[0:64, 0:P], an[:, P:DV], ident[:])
                        nc.vector.tensor_copy(attTb[:, iabs:iabs + P], tb[0:64, 0:P])

        # ---------------- phase 3: per-batch AllToAll + output projection ----------------
        # a2a[b]: input [8 dest, 192 dv, 256 rows of batch b] -> core c owns
        # batch-b rows [c*256,(c+1)*256).  out rows: [b0 256 | b1 256].
        HQ = NQ // B  # 256 rows per batch per core
        with tc.tile_pool(name="dram", bufs=1, space="DRAM") as dram, \
             tc.tile_pool(name="gat", bufs=1) as gpool, \
             tc.tile_pool(name="yp", bufs=4, space="PSUM") as ypsum, \
             tc.tile_pool(name="yo", bufs=4) as ypool:
            gatT = gpool.tile([P, DCH * NQ], BF16, tag="gatT")
            nc.sync.dma_start(out=wo_sb[:], in_=wo[:])
            nc.sync.dma_start(out=bor_sb[:], in_=bor[:])

            def do_exchange(b):
                bs = b * N
                a2a_in = dram.tile([NCORES * DV, HQ], BF16, name=f"a2a_in{b}", tag=f"a2a_in{b}")
                a2a_out = dram.tile([NCORES * DV, HQ], BF16, name=f"a2a_out{b}", tag=f"a2a_out{b}")
                nc.sync.dma_start(
                    out=a2a_in[:].rearrange("(d r) n -> r d n", r=DV)[0:P],
                    in_=attTa[:, bs:bs + N].rearrange("p (d n) -> p d n", d=NCORES))
                nc.sync.dma_start(
                    out=a2a_in[:].rearrange("(d r) n -> r d n", r=DV)[P:DV],
                    in_=attTb[:, bs:bs + N].rearrange("p (d n) -> p d n", d=NCORES))
                nc.gpsimd.collective_compute(
                    "AllToAll", AT.bypass,
                    replica_groups=[list(range(NCORES))],
                    ins=[a2a_in[:].opt()], outs=[a2a_out[:].opt()])
                # received [src, dv 192, my 256 rows] = [1536 dv, 256]
                nc.sync.dma_start(
                    out=gatT[:].rearrange("p (c n) -> p c n", c=DCH)[:, :, b * HQ:(b + 1) * HQ],
                    in_=a2a_out[:].rearrange("(c p) n -> p c n", p=P))

            def do_outproj(b, ypsum, ypool):
                for it in range(HQ // P):
                    row = b * HQ + it * P
                    for ot in range(DIM // IBLK):
                        yp = ypsum.tile([P, IBLK], F32, tag="yp")
                        for kc in range(DCH):
                            nc.tensor.matmul(
                                yp[:], gatT[:, kc * NQ + row:kc * NQ + row + P],
                                wo_sb[:, kc * DIM + ot * IBLK:kc * DIM + (ot + 1) * IBLK],
                                start=(kc == 0), stop=(kc == DCH - 1))
                        yo = ypool.tile([P, IBLK], F32, tag="yo")
                        nc.vector.tensor_add(yo[:], yp[:], bor_sb[:, ot * IBLK:(ot + 1) * IBLK])
                        nc.sync.dma_start(
                            out=out[row:row + P, ot * IBLK:(ot + 1) * IBLK], in_=yo[:])

            for b in range(B):
                do_exchange(b)
                do_outproj(b)


# revision 35
# speedup vs baseline: 1.1161x; 1.0326x over previous
"""Distributed Trainium2 kernel: relative-position multi-head attention.

B=2, N=2048, DIM=1536, H=8, DK=64, DV=192.

Sharding: one head per core, both batches (8 heads / 8 cores).  Each core
projects q/k/v for its head over all 4096 tokens, runs attention, transposes
its attention output to feature-major, then one 8-core AllToAll exchanges
token-slices: core c ends up with all 8 heads' outputs for flat token rows
[c*512,(c+1)*512) of the flattened [4096, DIM] output, and computes that
slice of the output projection.

Math: rel_k = distances @ W_rel is rank-1, so after relative_shift the
positional logits are s_i*(j-i) with s_i = (q_i*scale+rpb)@w_h.  The -s_i*i
term is constant per softmax row and drops under softmax.  So
logits = (q*scale+rcb)@k^T + s_i*j, realized as 2 extra contraction rows of
the QK^T matmul: [jvec, ones] on the K side and [s_i, -B_i] on the Q side,
where B_i upper-bounds the row max so exp cannot overflow; it cancels
exactly in softmax.  s_i comes free out of the projection via an extra
weight column u = Wq_scaled @ w_h.
"""

import contextlib

import ml_dtypes
import numpy as np

import concourse.bass as bass
import concourse.bacc as bacc_mod
import concourse.mybir as mybir
import concourse.tile as tile
from concourse.bass_utils import run_bass_kernel_spmd
from concourse.masks import make_identity

B, N, DIM, H, DK, DV = 2, 2048, 1536, 8, 64, 192
NCORES = 8
NT = B * N               # 4096 flat tokens
NQ = NT // NCORES        # 512 output rows per core
P = 128
DCH = DIM // P           # 12 projection contraction chunks
NTILE = N // P           # 16 token tiles per batch
IBLK = 512
NIB = N // IBLK          # 4 i-blocks per batch
F32 = mybir.dt.float32
F32R = mybir.dt.float32r
BF16 = mybir.dt.bfloat16
AT = mybir.AluOpType
AF = mybir.ActivationFunctionType
CONTENT_BOUND = 48.0

_CACHE = {}


def _build_body(nc, tc, xT, wqk, wv1, wv2, wo, krows, ccol, bor, out):
    ctx = contextlib.ExitStack()
    with ctx:
        persist = ctx.enter_context(tc.tile_pool(name="persist", bufs=1))

        wqk_sb = persist.tile([P, DCH * P], BF16, tag="wqk")
        wv1_sb = persist.tile([P, DCH * P], BF16, tag="wv1")
        wv2_sb = persist.tile([P, DCH * 65], BF16, tag="wv2")
        wo_sb = persist.tile([P, DCH * DIM], BF16, tag="wo")
        ccol_sb = persist.tile([P, 2], F32, tag="ccol")
        bor_sb = persist.tile([P, DIM], F32, tag="bor")
        ident = persist.tile([P, P], BF16, tag="ident")

        for w_sb, w_dram in ((wqk_sb, wqk), (wv1_sb, wv1), (wv2_sb, wv2)):
            nc.sync.dma_start(out=w_sb[:], in_=w_dram[:])
        nc.sync.dma_start(out=ccol_sb[:], in_=ccol[:])
        make_identity(nc, ident[:])

        # Q'/K' per flat token: rows 0-63 content, 64 ramp (s / jvec), 65 bias (-B / ones)
        QT = persist.tile([67, NT], BF16, tag="QT")
        KT = persist.tile([67, NT], BF16, tag="KT")
        # v token-major per (b, j-tile): [dv(192) | ones]
        vtok = persist.tile([P, B * NTILE * (DV + 1)], BF16, tag="vtok")
        # attention output feature-major (transposed), split 128/64 partitions
        attTa = persist.tile([P, NT], BF16, tag="attTa")
        attTb = persist.tile([64, NT], BF16, tag="attTb")

        for b in range(B):
            bs = b * N
            nc.sync.dma_start(out=KT[64:67, bs:bs + N], in_=krows[:])  # jhi; jlo; ones

        # ---------------- phase 1: projections ----------------
        with tc.tile_pool(name="xch", bufs=16) as xpool, \
             tc.tile_pool(name="prjp", bufs=2, space="PSUM") as ppsum, \
             tc.tile_pool(name="vT", bufs=1) as vtpool, \
             tc.tile_pool(name="vtp", bufs=2, space="PSUM") as vtpsum, \
             tc.tile_pool(name="srow", bufs=1) as spool, \
             tc.tile_pool(name="sdram", bufs=2, space="DRAM") as sdram:

            srow = [spool.tile([1, N], F32, tag=f"srow{b}", name=f"srow{b}") for b in range(B)]

            vTa = vtpool.tile([P, NT], BF16, tag="vTa")
            vTb = vtpool.tile([64, NT], BF16, tag="vTb")

            xc_cache = {}
            for b in range(B):
                for it in range(NIB):
                    bt = b * NIB + it
                    sl = slice(bt * IBLK, (bt + 1) * IBLK)
                    pqk = ppsum.tile([P, IBLK], F32, tag="pqk", name=f"pqk{bt}")
                    pv1 = ppsum.tile([P, IBLK], F32, tag="pv1", name=f"pv1{bt}")
                    pv2 = ppsum.tile([P, IBLK], F32, tag="pv2", name=f"pv2{bt}")
                    for c in range(DCH):
                        if it % 2 == 0:
                            xc2 = xpool.tile([P, 2 * IBLK], BF16, tag="xc", name=f"xc{bt}_{c}")
                            nc.sync.dma_start(
                                out=xc2[:], in_=xT[c * P:(c + 1) * P, bt * IBLK:(bt + 2) * IBLK])
                            xc_cache[c] = xc2
                        mk = dict(start=(c == 0), stop=(c == DCH - 1))
                        xr = xc_cache[c][:, (it % 2) * IBLK:(it % 2 + 1) * IBLK]
                        nc.tensor.matmul(pqk[:], wqk_sb[:, c * P:(c + 1) * P], xr, **mk)
                        nc.tensor.matmul(pv1[:], wv1_sb[:, c * P:(c + 1) * P], xr, **mk)
                        nc.tensor.matmul(pv2[0:65, :], wv2_sb[:, c * 65:(c + 1) * 65], xr, **mk)
                    nc.vector.tensor_scalar_add(QT[0:DK, sl], pqk[0:DK, :], ccol_sb[0:DK, 0:1])
                    nc.vector.tensor_copy(KT[0:DK, sl], pqk[DK:2 * DK, :])
                    nc.vector.tensor_copy(vTa[:, sl], pv1[:])
                    nc.vector.tensor_copy(vTb[:, sl], pv2[0:DK, :])
                    nc.vector.tensor_scalar_add(
                        srow[b][0:1, it * IBLK:(it + 1) * IBLK],
                        pv2[64:65, :], ccol_sb[0:1, 1:2])

                # assemble batch b immediately: v token-major + QT ramp rows
                bs = b * N
                for jt in range(NTILE):
                    bj = b * NTILE + jt
                    tp = vtpsum.tile([P, 2 * P], BF16, tag="vtp", name=f"vtp{bj}")
                    nc.tensor.transpose(tp[:, 0:P], vTa[:, bj * P:(bj + 1) * P], ident[:])
                    nc.tensor.transpose(tp[:, P:P + 64], vTb[:, bj * P:(bj + 1) * P], ident[0:64, 0:64])
                    base = bj * (DV + 1)
                    nc.vector.tensor_copy(vtok[:, base:base + P], tp[:, 0:P])
                    nc.vector.tensor_copy(vtok[:, base + P:base + DV], tp[:, P:P + 64])
                    nc.gpsimd.memset(vtok[:, base + DV:base + DV + 1], 1.0)
                # B = relu(s)*(N-1) + CONTENT_BOUND  (ramp max over j; -s*i dropped)
                t8 = spool.tile([1, N], BF16, tag="bt8", name=f"bt8{b}")
                t1f = spool.tile([1, N], BF16, tag="bt1f", name=f"bt1f{b}")
                tb = spool.tile([1, N], BF16, tag="btb", name=f"btb{b}")
                tr = spool.tile([1, N], F32, tag="btr", name=f"btr{b}")
                nc.vector.tensor_scalar_mul(t8[:], srow[b][:], 8.0)
                nc.vector.tensor_copy(t1f[:], srow[b][:])
                nc.vector.tensor_scalar_max(tr[:], srow[b][:], 0.0)
                nc.vector.tensor_scalar(tb[:], tr[:], -float(N - 1), -CONTENT_BOUND, AT.mult, AT.add)
                qs3 = sdram.tile([3, N], BF16, tag="qs3", name=f"qs3{b}")
                nc.sync.dma_start(out=qs3[0:1, :], in_=t8[:])
                nc.sync.dma_start(out=qs3[1:2, :], in_=t1f[:])
                nc.sync.dma_start(out=qs3[2:3, :], in_=tb[:])
                nc.sync.dma_start(out=QT[64:67, bs:bs + N], in_=qs3[:])

        # ---------------- phase 2: attention ----------------
        with tc.tile_pool(name="et", bufs=2) as epool, \
             tc.tile_pool(name="lg", bufs=3, space="PSUM") as lgp, \
             tc.tile_pool(name="av", bufs=3, space="PSUM") as avp, \
             tc.tile_pool(name="atp", bufs=1, space="PSUM") as atpp, \
             tc.tile_pool(name="rz", bufs=4) as rzpool, \
             tc.tile_pool(name="an", bufs=4) as anpool:
            for b in range(B):
                bs = b * N
                for ib in range(NIB):
                    isl = slice(bs + ib * IBLK, bs + (ib + 1) * IBLK)
                    eT = epool.tile([P, NTILE * IBLK], BF16, tag="eT")
                    for jt in range(NTILE):
                        lg = lgp.tile([P, IBLK], F32, tag="lg")
                        nc.tensor.matmul(
                            lg[:], KT[:, bs + jt * P:bs + (jt + 1) * P],
                            QT[:, isl], start=True, stop=True)
                        nc.scalar.activation(eT[:, jt * IBLK:(jt + 1) * IBLK], lg[:], AF.Exp)
                    for ic in range(IBLK // P):
                        av = avp.tile([P, DV + 1], F32, tag="av")
                        for jt in range(NTILE):
                            nc.tensor.matmul(
                                av[:],
                                eT[:, jt * IBLK + ic * P:jt * IBLK + (ic + 1) * P],
                                vtok[:, (b * NTILE + jt) * (DV + 1):(b * NTILE + jt + 1) * (DV + 1)],
                                start=(jt == 0), stop=(jt == NTILE - 1))
                        rz = rzpool.tile([P, 1], F32, tag="rz")
                        nc.vector.reciprocal(rz[:], av[:, DV:DV + 1])
                        an = anpool.tile([P, DV], BF16, tag="an")
                        nc.vector.tensor_scalar_mul(an[:], av[:, 0:DV], rz[:])
                        # transpose [i, dv] -> feature-major attTa/attTb
                        iabs = bs + ib * IBLK + ic * P
                        ta = atpp.tile([P, P], BF16, tag="ta")
                        nc.tensor.transpose(ta[:], an[:, 0:P], ident[:])
                        nc.vector.tensor_copy(attTa[:, iabs:iabs + P], ta[:])
                        tb = atpp.tile([P, P], BF16, tag="tb")
                        nc.tensor.transpose(tb[0:64, 0:P], an[:, P:DV], ident[:])
                        nc.vector.tensor_copy(attTb[:, iabs:iabs + P], tb[0:64, 0:P])

        # ---------------- phase 3: per-batch AllToAll + output projection ----------------
        # a2a[b]: input [8 dest, 192 dv, 256 rows of batch b] -> core c owns
        # batch-b rows [c*256,(c+1)*256).  out rows: [b0 256 | b1 256].
        HQ = NQ // B  # 256 rows per batch per core
        with tc.tile_pool(name="dram", bufs=1, space="DRAM") as dram, \
             tc.tile_pool(name="gat", bufs=1) as gpool, \
             tc.tile_pool(name="yp", bufs=4, space="PSUM") as ypsum, \
             tc.tile_pool(name="yo", bufs=4) as ypool:
            gatT = gpool.tile([P, DCH * NQ], BF16, tag="gatT")
            nc.sync.dma_start(out=wo_sb[:], in_=wo[:])
            nc.sync.dma_start(out=bor_sb[:], in_=bor[:])

            def do_exchange(b):
                bs = b * N
                a2a_in = dram.tile([NCORES * DV, HQ], BF16, name=f"a2a_in{b}", tag=f"a2a_in{b}")
                a2a_out = dram.tile([NCORES * DV, HQ], BF16, name=f"a2a_out{b}", tag=f"a2a_out{b}")
                nc.sync.dma_start(
                    out=a2a_in[:].rearrange("(d r) n -> r d n", r=DV)[0:P],
                    in_=attTa[:, bs:bs + N].rearrange("p (d n) -> p d n", d=NCORES))
                nc.sync.dma_start(
                    out=a2a_in[:].rearrange("(d r) n -> r d n", r=DV)[P:DV],
                    in_=attTb[:, bs:bs + N].rearrange("p (d n) -> p d n", d=NCORES))
                nc.gpsimd.collective_compute(
                    "AllToAll", AT.bypass,
                    replica_groups=[list(range(NCORES))],
                    ins=[a2a_in[:].opt()], outs=[a2a_out[:].opt()])
                # received [src, dv 192, my 256 rows] = [1536 dv, 256]
                nc.sync.dma_start(
                    out=gatT[:].rearrange("p (c n) -> p c n", c=DCH)[:, :, b * HQ:(b + 1) * HQ],
                    in_=a2a_out[:].rearrange("(c p) n -> p c n", p=P))

            def do_outproj(b, ypsum, ypool):
                for it in range(HQ // P):
                    row = b * HQ + it * P
                    for ot in range(DIM // IBLK):
                        yp = ypsum.tile([P, IBLK], F32, tag="yp")
                        for kc in range(DCH):
                            nc.tensor.matmul(
                                yp[:], gatT[:, kc * NQ + row:kc * NQ + row + P],
                                wo_sb[:, kc * DIM + ot * IBLK:kc * DIM + (ot + 1) * IBLK],
                                start=(kc == 0), stop=(kc == DCH - 1))
                        yo = ypool.tile([P, IBLK], F32, tag="yo")
                        nc.vector.tensor_add(yo[:], yp[:], bor_sb[:, ot * IBLK:(ot + 1) * IBLK])
                        nc.sync.dma_start(
                            out=out[row:row + P, ot * IBLK:(ot + 1) * IBLK], in_=yo[:])

            for b in range(B):
                do_exchange(b)
                do_outproj(b)
